# revision 36
# baseline (speedup 1.0000x reference)
"""Trainium2 Bass kernel for nn_Attention_83081847374268 (sparse sliding-window GQA).

Sharding: 8 cores = batch (2, data parallel) x kv-head (4, tensor parallel).
Each core computes, for its (b, kh): q/k/v projections (2 q heads, 1 kv head),
QK-RMSNorm + RoPE, banded sliding-window attention, and a partial output
projection against its 512-row slice of wout.  The host sums the 4 partials
per batch (the TP reduction) and stacks the batches.

v2 design (all matmul operands bf16, fp32 PSUM accumulation):
  stage A: stream xT column-chunks; q/k projections in transposed layout
           (qT/kT [head_dim, T]); RMSNorm via ones-matmul variance +
           PE-broadcast rstd; RoPE fused with the rstd multiply on DVE;
           v projected in NATURAL layout [T, 256] via lhsT=x chunks (no
           transposes, no DRAM bounce).
  stage B: per 128-query tile, for each 128-key chunk of the band:
           S^T[keys, 2hx128q] = kT_chunk^T @ qT  (so exp gives P^T directly,
           no PE transposes); masked chunks multiplied by small static 0/1
           patterns; denominator via ones-matmul over partitions; PV
           accumulates enc^T; normalization fused into the PSUM->SBUF move.
  stage C: out partial = encT.T @ wout_slice, interleaved one tile behind
           stage B; yp written as bf16 (host sums partials in fp64).
"""
import sys

sys.path.insert(0, "/opt/trn_rl_repo")

import numpy as np
import ml_dtypes

import concourse.bacc as bacc
import concourse.mybir as mybir
from concourse.bass_utils import run_bass_kernel_spmd
from concourse.tile import TileContext
from concourse.alu_op_type import AluOpType

F32 = mybir.dt.float32
F32R = mybir.dt.float32r
BF16 = mybir.dt.bfloat16
ACTF = mybir.ActivationFunctionType

B, T, WIDTH = 2, 2048, 2048
NUM_HEADS, NUM_KV_HEADS, HEAD_DIM = 8, 4, 256
GROUPS = NUM_HEADS // NUM_KV_HEADS  # 2 q heads per kv head (= per core)
WINDOW = 512
ROPE_BASE = 10000.0
ALPHA = HEAD_DIM ** -0.5

NT = T // 128           # 16 query tiles
TCH = 512               # stage-A t-chunk width
NTCH = T // TCH         # 4
NW = WIDTH // 128       # 16 contraction chunks

_prog_cache = {}
DEBUG_TAPS = False


def _geometry(positions, attn_mask):
    """Per-(tile, chunk) structure from the actual mask/positions data.

    Returns (chunk_lists, slot_of, masks_per_batch):
      chunk_lists[it] = list of 128-key chunk indices with any valid key
                        (union over batches, so the SPMD program is shared)
      slot_of[(it, c)] = mask-pattern slot, or None if fully valid
      masks_per_batch[b] = [n_slots, 128, 256] float32 0/1 multiplicative
                        masks (pattern doubled along axis 1 for the 2 heads)
    """
    pos = np.asarray(positions)
    am = np.asarray(attn_mask)
    pd = pos[:, :, None].astype(np.int64) - pos[:, None, :].astype(np.int64)
    valid = am & (np.abs(pd) < WINDOW)  # [B, T, T] bool
    assert valid.any(axis=2).all(), "a query row with no valid key is unsupported"

    chunk_lists = []
    slot_of = {}
    joint_slots = {}
    slot_blocks = []  # list of [B, 128, 128] bool
    for it in range(NT):
        qs = slice(it * 128, (it + 1) * 128)
        cs = []
        for c in range(NT):
            blk = valid[:, qs, c * 128:(c + 1) * 128]  # [B, 128, 128]
            if not blk.any():
                continue
            cs.append(c)
            if blk.all():
                slot_of[(it, c)] = None
            else:
                key = blk.tobytes()
                if key not in joint_slots:
                    joint_slots[key] = len(slot_blocks)
                    slot_blocks.append(blk)
                slot_of[(it, c)] = joint_slots[key]
        chunk_lists.append(tuple(cs))
    n_slots = max(1, len(slot_blocks))
    assert len(slot_blocks) <= 16, f"too many mask patterns: {len(slot_blocks)}"
    masks = np.ones((B, n_slots, 128, 256), np.float32)
    for s, blk in enumerate(slot_blocks):
        # patterns are collected as [q, key] blocks but applied to the
        # transposed P^T [key, q] tiles -> transpose, doubled for 2 heads
        m = blk.astype(np.float32).transpose(0, 2, 1)  # [B, 128k, 128q]
        masks[:, s, :, 0:128] = m
        masks[:, s, :, 128:256] = m
    return tuple(chunk_lists), slot_of, masks


def _rope_tables(pos_b, scale):
    """cos/sin tables in [head_dim/2, T] (transposed) layout, gain folded in."""
    d = np.arange(HEAD_DIM // 2, dtype=np.float32)
    timescale = (ROPE_BASE ** (2.0 / HEAD_DIM * d)).astype(np.float32)
    rad = pos_b.astype(np.float32)[None, :] / timescale[:, None]  # [128, T]
    cos, sin = np.cos(rad).astype(np.float32), np.sin(rad).astype(np.float32)
    g1 = (1.0 + scale[:HEAD_DIM // 2]).astype(np.float32)[:, None]
    g2 = (1.0 + scale[HEAD_DIM // 2:]).astype(np.float32)[:, None]
    # o1 = a1*C1 - a2*S2 ; o2 = a2*C2 + a1*S1
    return (cos * g1, sin * g1, cos * g2, sin * g2)  # C1, S1, C2, S2


def _build(chunk_lists, slot_items, n_slots, shared_tables, debug_taps=False):
    slot_of = dict(slot_items)
    nc = bacc.Bacc("TRN2", target_bir_lowering=False, debug=False, num_devices=8)

    def din(name, shape, dt):
        return nc.dram_tensor(name, shape, dt, kind="ExternalInput").ap()

    xT = din("xT", [WIDTH, T], BF16)
    wq = din("wq", [WIDTH, 512], BF16)
    wk = din("wk", [WIDTH, 256], BF16)
    wv = din("wv", [WIDTH, 256], BF16)
    wout = din("wout", [512, T], BF16)
    ones1_r = din("ones1", [1, 128], F32R)   # K=1 broadcast lhsT
    onesc_r = din("onesc", [128, 1], F32R)   # partition-sum lhsT (f32r)
    onesc_b = din("onescb", [128, 1], BF16)  # partition-sum lhsT (bf16)
    masks_d = din("masks", [n_slots, 128, 256], BF16)
    tab_names = ["ct", "st"] if shared_tables else [
        "cq1", "sq1", "cq2", "sq2", "ck1", "sk1", "ck2", "sk2"]
    tabs = {n: din(n, [128, T], F32) for n in tab_names}
    yp = nc.dram_tensor("yp", [T, T], BF16, kind="ExternalOutput").ap()

    taps = {}
    if debug_taps:
        for nm, shape in (("qT_tap", [2, 128, NT * 256]), ("kT_tap", [2, 128, T]),
                          ("v_tap", [128, NT * 256]), ("encT_tap", [4, 128, T]),
                          ("pts_tap", [128, 5 * 256])):
            taps[nm] = nc.dram_tensor(nm, shape, BF16, kind="ExternalOutput").ap()

    with TileContext(nc) as tc:
        with (
            tc.tile_pool(name="persist", bufs=1) as pp,
            tc.tile_pool(name="qk_store", bufs=1) as qkp,
        ):
            ones1 = pp.tile([1, 128], F32R)
            nc.sync.dma_start(out=ones1[:], in_=ones1_r[:])
            onescr = pp.tile([128, 1], F32R)
            nc.sync.dma_start(out=onescr[:], in_=onesc_r[:])
            onescb = pp.tile([128, 1], BF16)
            nc.sync.dma_start(out=onescb[:], in_=onesc_b[:])
            masks_sb = pp.tile([128, n_slots * 256], BF16)
            epsb = pp.tile([1, 1], F32)
            nc.any.memset(epsb[:], 1e-6)
            epsbq = pp.tile([1, 1], F32)
            nc.any.memset(epsbq[:], HEAD_DIM * 1e-6)

            # qT layout per cc: [128, (it, h, 128)] so stage-B rhs slices
            # [it*256, it*256+256) pack both q heads contiguously.
            qT = [qkp.tile([128, NT * 256], BF16, tag=f"qT{c}", name=f"qT{c}")
                  for c in range(2)]
            kT = [qkp.tile([128, T], BF16, tag=f"kT{c}", name=f"kT{c}")
                  for c in range(2)]
            v_sb = qkp.tile([128, NT * 256], BF16, tag="vsb", name="vsb")
            encT = [qkp.tile([128, T], BF16, tag=f"encT{j}", name=f"encT{j}")
                    for j in range(4)]
            wout_sb = [qkp.tile([128, T], BF16, tag=f"wo{j}", name=f"wo{j}")
                       for j in range(4)]
            wout_r = wout.rearrange("(c p) t -> c p t", p=128)

            # ---------------- stage A: projections + RMSNorm + RoPE ----------
            with (
                tc.tile_pool(name="wpool", bufs=1) as wp,
                tc.tile_pool(name="xpool", bufs=2) as xp,
                tc.tile_pool(name="tabpool", bufs=2) as tp,
                tc.tile_pool(name="sa", bufs=2) as sa,
                tc.tile_pool(name="psA", bufs=2, space="PSUM") as psA,
                tc.tile_pool(name="psAux", bufs=2, space="PSUM") as psAux,
                tc.tile_pool(name="psAux1", bufs=1, space="PSUM") as psAux1,
            ):
                wq_t = wp.tile([128, NW * 512], BF16)
                wk_t = wp.tile([128, NW * 256], BF16)
                wv_t = wp.tile([128, NW * 256], BF16)
                wq_r = wq.rearrange("(c p) m -> p c m", p=128)
                wk_r = wk.rearrange("(c p) m -> p c m", p=128)
                wv_r = wv.rearrange("(c p) m -> p c m", p=128)
                wq_v = wq_t[:].rearrange("p (c m) -> p c m", m=512)
                wk_v = wk_t[:].rearrange("p (c m) -> p c m", m=256)
                wv_v = wv_t[:].rearrange("p (c m) -> p c m", m=256)
                xT_r = xT.rearrange("(c p) t -> p c t", p=128)

                def load_xts(tci):
                    t0 = tci * TCH
                    xts = xp.tile([128, NW * TCH], BF16, tag="xts", name=f"xts{tci}")
                    xv = xts[:].rearrange("p (c t) -> p c t", t=TCH)
                    for q4 in range(4):
                        nc.sync.dma_start(
                            out=xv[:, q4 * 4:(q4 + 1) * 4],
                            in_=xT_r[:, q4 * 4:(q4 + 1) * 4, t0:t0 + TCH],
                        )
                    return xts

                xts_pre = xp.tile([128, NW * TCH], BF16, tag="xts", name="xts0")
                xv0 = xts_pre[:].rearrange("p (c t) -> p c t", t=TCH)
                for q4 in range(4):
                    nc.sync.dma_start(out=wq_v[:, q4 * 4:(q4 + 1) * 4],
                                      in_=wq_r[:, q4 * 4:(q4 + 1) * 4])
                    nc.sync.dma_start(out=xv0[:, q4 * 4:(q4 + 1) * 4],
                                      in_=xT_r[:, q4 * 4:(q4 + 1) * 4, 0:TCH])
                for q4 in range(4):
                    nc.sync.dma_start(out=wk_v[:, q4 * 4:(q4 + 1) * 4],
                                      in_=wk_r[:, q4 * 4:(q4 + 1) * 4])

                if shared_tables:
                    q_tabs = k_tabs = ("ct", "st", "ct", "st")
                else:
                    q_tabs = ("cq1", "sq1", "cq2", "sq2")
                    k_tabs = ("ck1", "sk1", "ck2", "sk2")
                # (w_tile, wcols, col_base, table_keys, is_q, qhead)
                # q-head1 last: its final-chunk rope runs on GPSIMD after
                # stage A ends and only gates stage-B tiles 12-15
                units = [
                    (wq_t, 512, 0, q_tabs, True, 0),
                    (wk_t, 256, 0, k_tabs, False, None),
                    (wq_t, 512, 256, q_tabs, True, 1),
                ]

                def emit_v(tci, s4, xts):
                    t0 = tci * TCH
                    psv = psAux.tile([128, 256], F32, tag="pv",
                                     name=f"psv{tci}_{s4}")
                    for wc in range(NW):
                        nc.tensor.matmul(
                            psv[:],
                            xts[:, wc * TCH + s4 * 128: wc * TCH + (s4 + 1) * 128],
                            wv_t[:, wc * 256:(wc + 1) * 256],
                            start=(wc == 0), stop=(wc == NW - 1),
                        )
                    it = tci * 4 + s4
                    nc.scalar.activation(v_sb[:, it * 256:(it + 1) * 256],
                                         psv[:], ACTF.Copy)

                def emit_aux(pend):
                    """Variance -> rstd broadcast -> RoPE for a finished unit."""
                    (tci, ps1, ps2, tabt, tkeys, is_q, qhead) = pend
                    t0 = tci * TCH
                    sq1 = sa.tile([128, TCH], F32R, tag="sq1")
                    sq2 = sa.tile([128, TCH], F32R, tag="sq2")
                    nc.scalar.activation(sq1[:], ps1[:], ACTF.Square)
                    nc.scalar.activation(sq2[:], ps2[:], ACTF.Square)
                    psvar = psAux1.tile([1, TCH], F32, tag="var")
                    nc.tensor.matmul(psvar[:], onescr[:], sq1[:], start=True, stop=False)
                    nc.tensor.matmul(psvar[:], onescr[:], sq2[:], start=False, stop=True)
                    stdv = sa.tile([1, TCH], F32R, tag="stdv")
                    if is_q:
                        # fold alpha: stdv = sqrt(sum(q^2) + 256*eps) = 16*sqrt(var+eps)
                        nc.scalar.activation(stdv[:], psvar[:], ACTF.Sqrt,
                                             bias=epsbq[:])
                    else:
                        nc.scalar.activation(stdv[:], psvar[:], ACTF.Sqrt,
                                             scale=1.0 / HEAD_DIM, bias=epsb[:])
                    # evacuate the projection PSUM banks promptly via ACT so
                    # the next pipeline stage (and stage B's pools at the A->B
                    # boundary) can claim them without waiting on DVE's tail
                    cs1 = sa.tile([128, TCH], F32, tag="cs1")
                    cs2 = sa.tile([128, TCH], F32, tag="cs2")
                    nc.scalar.activation(cs1[:], ps1[:], ACTF.Copy)
                    nc.scalar.activation(cs2[:], ps2[:], ACTF.Copy)
                    psb = psAux1.tile([128, TCH], F32, tag="bc")
                    nc.tensor.matmul(psb[:], ones1[:], stdv[:], start=True, stop=True)
                    rb = sa.tile([128, TCH], F32, tag="rb")
                    nc.vector.reciprocal_approx_fast(out=rb[:], in_=psb[:])
                    # the very last unit's rope runs post-stage-A with nothing
                    # to hide it on DVE; GPSIMD is idle then (SBUF-only ops)
                    eng = (nc.gpsimd if (tci == NTCH - 1 and qhead == 1)
                           else nc.vector)
                    a1 = sa.tile([128, TCH], F32, tag="a1")
                    a2 = sa.tile([128, TCH], F32, tag="a2")
                    eng.tensor_tensor(a1[:], cs1[:], rb[:], AluOpType.mult)
                    eng.tensor_tensor(a2[:], cs2[:], rb[:], AluOpType.mult)
                    C1, S1, C2, S2 = (tabt[k] for k in tkeys)
                    m1 = sa.tile([128, TCH], F32, tag="m1")
                    m2 = sa.tile([128, TCH], F32, tag="m2")
                    m3 = sa.tile([128, TCH], F32, tag="m1", name="m3t")
                    m4 = sa.tile([128, TCH], F32, tag="m2", name="m4t")
                    eng.tensor_tensor(m1[:], a1[:], C1[:], AluOpType.mult)
                    eng.tensor_tensor(m2[:], a2[:], S2[:], AluOpType.mult)
                    eng.tensor_tensor(m3[:], a2[:], C2[:], AluOpType.mult)
                    eng.tensor_tensor(m4[:], a1[:], S1[:], AluOpType.mult)
                    if is_q:
                        qv1 = qT[0][:].rearrange("p (i m) -> p i m", m=256)
                        qv2 = qT[1][:].rearrange("p (i m) -> p i m", m=256)
                        d1 = qv1[:, tci * 4:(tci + 1) * 4, qhead * 128:(qhead + 1) * 128]
                        d2 = qv2[:, tci * 4:(tci + 1) * 4, qhead * 128:(qhead + 1) * 128]
                    else:
                        d1 = kT[0][:].rearrange("p (i m) -> p i m", m=128)[
                            :, tci * 4:(tci + 1) * 4, :]
                        d2 = kT[1][:].rearrange("p (i m) -> p i m", m=128)[
                            :, tci * 4:(tci + 1) * 4, :]
                    m1v = m1[:].rearrange("p (i m) -> p i m", m=128)
                    m2v = m2[:].rearrange("p (i m) -> p i m", m=128)
                    m3v = m3[:].rearrange("p (i m) -> p i m", m=128)
                    m4v = m4[:].rearrange("p (i m) -> p i m", m=128)
                    eng.tensor_tensor(d1, m1v, m2v, AluOpType.subtract)
                    eng.tensor_tensor(d2, m3v, m4v, AluOpType.add)

                pend = None
                for tci in range(NTCH):
                    t0 = tci * TCH
                    xts = xts_pre if tci == 0 else load_xts(tci)
                    tabt = {}
                    for name in dict.fromkeys(q_tabs + k_tabs):
                        tt = tp.tile([128, TCH], F32, tag=name, name=f"tab_{name}")
                        nc.sync.dma_start(out=tt[:], in_=tabs[name][:, t0:t0 + TCH])
                        tabt[name] = tt
                    if tci == 0:
                        for q4 in range(4):
                            nc.sync.dma_start(out=wv_v[:, q4 * 4:(q4 + 1) * 4],
                                              in_=wv_r[:, q4 * 4:(q4 + 1) * 4])
                        # stage-B/C-only data: queued after the stage-A kickoff
                        # so it doesn't delay the first matmuls
                        nc.sync.dma_start(
                            out=masks_sb[:].rearrange("p (s m) -> p s m", m=256),
                            in_=masks_d.rearrange("s p m -> p s m"))
                        for j in range(4):
                            nc.sync.dma_start(out=wout_sb[j][:], in_=wout_r[j])
                    for ui, (w_t, wcols, cbase, tkeys, is_q, qhead) in enumerate(units):
                        ps1 = psA.tile([128, TCH], F32, tag="s0")
                        ps2 = psA.tile([128, TCH], F32, tag="s1")
                        for ps, cc in ((ps1, 0), (ps2, 1)):
                            coff = cbase + cc * 128
                            for wc in range(NW):
                                nc.tensor.matmul(
                                    ps[:],
                                    w_t[:, wc * wcols + coff: wc * wcols + coff + 128],
                                    xts[:, wc * TCH:(wc + 1) * TCH],
                                    start=(wc == 0), stop=(wc == NW - 1),
                                )
                        # v-projection matmuls act as PE filler while ACT/DVE
                        # work through the previous unit's variance/rope chain.
                        emit_v(tci, ui, xts)
                        if pend is not None:
                            emit_aux(pend)
                        pend = (tci, ps1, ps2, tabt, tkeys, is_q, qhead)
                    emit_v(tci, 3, xts)
                emit_aux(pend)

            if debug_taps:
                for c in range(2):
                    nc.sync.dma_start(out=taps["qT_tap"][c], in_=qT[c][:])
                    nc.sync.dma_start(out=taps["kT_tap"][c], in_=kT[c][:])
                nc.sync.dma_start(out=taps["v_tap"][:], in_=v_sb[:])

            # ---------------- stage B + C: banded attention + out proj -------
            with (
                tc.tile_pool(name="ptsp", bufs=2) as ptsp,
                tc.tile_pool(name="sb", bufs=2) as sbp,
                tc.tile_pool(name="outp", bufs=4) as outp,
                tc.tile_pool(name="psS", bufs=3, space="PSUM") as psS,
                tc.tile_pool(name="psDen", bufs=1, space="PSUM") as psDen,
                tc.tile_pool(name="psE", bufs=2, space="PSUM") as psE,
                tc.tile_pool(name="psO", bufs=2, space="PSUM") as psO,
            ):
                def emit_C(it):
                    for nb in range(4):
                        ops = psO.tile([128, 512], F32, tag="o", name=f"o{it}_{nb}")
                        for j in range(4):
                            nc.tensor.matmul(
                                ops[:],
                                encT[j][:, it * 128:(it + 1) * 128],
                                wout_sb[j][:, nb * 512:(nb + 1) * 512],
                                start=(j == 0), stop=(j == 3),
                            )
                        ob = outp.tile([128, 512], BF16, tag="ob", name=f"ob{it}_{nb}")
                        if nb % 2 == 0:
                            nc.scalar.activation(ob[:], ops[:], ACTF.Copy)
                        else:
                            nc.vector.tensor_copy(ob[:], ops[:])
                        nc.sync.dma_start(
                            out=yp[it * 128:(it + 1) * 128, nb * 512:(nb + 1) * 512],
                            in_=ob[:])

                pendC = None
                pendN = None  # (it, pse, den_s): psrb+normalize deferred 1 tile

                def finalize(itN, pseN, den_sN):
                    nonlocal pendC
                    # psrb borrows a psO slot so it never contends with the
                    # den accumulator's single bank
                    psrb = psO.tile([128, 256], F32, tag="o", name=f"rb{itN}")
                    nc.tensor.matmul(psrb[:], ones1[:], den_sN[:],
                                     start=True, stop=True)
                    rdenb = sbp.tile([128, 256], F32, tag="rdenb",
                                     name=f"rdb{itN}")
                    nc.vector.reciprocal_approx_fast(out=rdenb[:], in_=psrb[:])
                    for cc in range(2):
                        for h in range(2):
                            nc.vector.tensor_tensor(
                                encT[2 * h + cc][:, itN * 128:(itN + 1) * 128],
                                pseN[:, cc * 256 + h * 128: cc * 256 + (h + 1) * 128],
                                rdenb[:, h * 128:(h + 1) * 128],
                                AluOpType.mult)
                    pendC = itN

                def tile_body(it):
                    nonlocal pendC, pendN
                    cs = chunk_lists[it]
                    ncs = len(cs)
                    pts = ptsp.tile([128, ncs * 256], BF16, tag="pts", name=f"pts{it}")
                    # S^T per chunk + exp (+ mask)
                    for ci, c in enumerate(cs):
                        pss = psS.tile([128, 256], F32, tag="s", name=f"s{it}_{ci}")
                        for cc in range(2):
                            nc.tensor.matmul(
                                pss[:],
                                kT[cc][:, c * 128:(c + 1) * 128],
                                qT[cc][:, it * 256:(it + 1) * 256],
                                start=(cc == 0), stop=(cc == 1),
                            )
                        slot = slot_of[(it, c)]
                        dst = pts[:, ci * 256:(ci + 1) * 256]
                        if slot is None:
                            nc.scalar.activation(dst, pss[:], ACTF.Exp)
                        else:
                            ptmp = sbp.tile([128, 256], BF16, tag="ptmp",
                                            name=f"pt{it}_{ci}")
                            nc.scalar.activation(ptmp[:], pss[:], ACTF.Exp)
                            # GPSIMD (idle otherwise) so the DVE queue never
                            # gates the denominator matmuls
                            nc.gpsimd.tensor_tensor(
                                dst, ptmp[:],
                                masks_sb[:, slot * 256:(slot + 1) * 256],
                                AluOpType.mult)
                    # stage-C matmuls of the previous tile fill the PE queue
                    # while ACT finishes the exps this tile's den/PV need.
                    if pendC is not None:
                        emit_C(pendC)
                    # denominator (partition-sum of P^T over all chunks)
                    psd = psDen.tile([1, 256], F32, tag="dn", name=f"d{it}")
                    for ci in range(ncs):
                        nc.tensor.matmul(
                            psd[:], onescb[:], pts[:, ci * 256:(ci + 1) * 256],
                            start=(ci == 0), stop=(ci == ncs - 1),
                        )
                    den_s = sbp.tile([1, 256], F32R, tag="dens", name=f"dens{it}")
                    nc.scalar.activation(den_s[:], psd[:], ACTF.Copy)
                    # PV accumulation (both head-dim halves in one bank-wide
                    # tile so psE can double-buffer within the bank budget)
                    pse = psE.tile([128, 512], F32, tag="e", name=f"e{it}")
                    for cc in range(2):
                        for ci, c in enumerate(cs):
                            nc.tensor.matmul(
                                pse[:, cc * 256:(cc + 1) * 256],
                                v_sb[:, c * 256 + cc * 128: c * 256 + (cc + 1) * 128],
                                pts[:, ci * 256:(ci + 1) * 256],
                                start=(ci == 0), stop=(ci == ncs - 1),
                            )
                    # psrb + normalize for the PREVIOUS tile: by now its
                    # den_s has long cleared the ACT queue, so no PE stall
                    if pendN is not None:
                        finalize(*pendN)
                    if debug_taps and it == 8:
                        nc.sync.dma_start(out=taps["pts_tap"][:], in_=pts[:])
                    pendN = (it, pse, den_s)

                for it in range(NT):
                    tile_body(it)
                if pendC is not None:
                    emit_C(pendC)
                finalize(*pendN)
                emit_C(pendC)

                if debug_taps:
                    for j in range(4):
                        nc.sync.dma_start(out=taps["encT_tap"][j], in_=encT[j][:])

    nc.compile()
    return nc


def kernel(x, positions, attn_mask, wq, wkv, wout, q_scale, k_scale):
    x = np.ascontiguousarray(x, np.float32)
    positions = np.asarray(positions)
    wq = np.ascontiguousarray(wq, np.float32)
    wkv = np.ascontiguousarray(wkv, np.float32)
    wout = np.ascontiguousarray(wout, np.float32)
    q_scale = np.asarray(q_scale, np.float32)
    k_scale = np.asarray(k_scale, np.float32)

    chunk_lists, slot_of, masks = _geometry(positions, attn_mask)
    n_slots = masks.shape[1]
    shared = not (q_scale.any() or k_scale.any())

    key = (chunk_lists, tuple(sorted(slot_of.items(), key=lambda kv: kv[0])),
           n_slots, shared, DEBUG_TAPS)
    if key not in _prog_cache:
        _prog_cache[key] = _build(chunk_lists, key[1], n_slots, shared, DEBUG_TAPS)
    nc = _prog_cache[key]

    bf = ml_dtypes.bfloat16
    ones1 = np.ones((1, 128), np.float32)
    onesc = np.ones((128, 1), np.float32)

    in_maps = []
    for core in range(8):
        b, kh = divmod(core, NUM_KV_HEADS)
        m = {
            "xT": np.ascontiguousarray(x[b].T).astype(bf),
            "wq": np.ascontiguousarray(wq[:, kh * 512:(kh + 1) * 512]).astype(bf),
            "wk": np.ascontiguousarray(wkv[:, kh * 256:(kh + 1) * 256]).astype(bf),
            "wv": np.ascontiguousarray(
                wkv[:, 1024 + kh * 256: 1024 + (kh + 1) * 256]).astype(bf),
            "wout": np.ascontiguousarray(wout[kh * 512:(kh + 1) * 512, :]).astype(bf),
            "ones1": ones1, "onesc": onesc, "onescb": onesc.astype(bf),
            "masks": masks[b].astype(bf),
        }
        if shared:
            ct, st, _, _ = _rope_tables(positions[b], np.zeros(HEAD_DIM, np.float32))
            m["ct"], m["st"] = ct, st
        else:
            for nm, tb in zip(("cq1", "sq1", "cq2", "sq2"),
                              _rope_tables(positions[b], q_scale)):
                m[nm] = tb
            for nm, tb in zip(("ck1", "sk1", "ck2", "sk2"),
                              _rope_tables(positions[b], k_scale)):
                m[nm] = tb
        in_maps.append(m)

    res = run_bass_kernel_spmd(nc, in_maps, list(range(8)))
    kernel._last_results = res
    out = np.empty((B, T, T), np.float32)
    for b in range(B):
        acc = res.results[b * NUM_KV_HEADS]["yp"].astype(np.float64)
        for kh in range(1, NUM_KV_HEADS):
            acc += res.results[b * NUM_KV_HEADS + kh]["yp"].astype(np.float64)
        out[b] = acc.astype(np.float32)
    return out


# revision 37
# speedup vs baseline: 1.2220x; 1.2220x over previous
"""Trainium2 Bass kernel for nn_Attention_83081847374268 (sparse sliding-window GQA).

Sharding: 8 cores = batch (2, data parallel) x kv-head (4, tensor parallel).
Each core computes, for its (b, kh): q/k/v projections (2 q heads, 1 kv head),
QK-RMSNorm + RoPE, banded sliding-window attention, and a partial output
projection against its 512-row slice of wout.  The host sums the 4 partials
per batch (the TP reduction) and stacks the batches.

v2 design (all matmul operands bf16, fp32 PSUM accumulation):
  stage A: stream xT column-chunks; q/k projections in transposed layout
           (qT/kT [head_dim, T]); RMSNorm via ones-matmul variance +
           PE-broadcast rstd; RoPE fused with the rstd multiply on DVE;
           v projected in NATURAL layout [T, 256] via lhsT=x chunks (no
           transposes, no DRAM bounce).
  stage B: per 128-query tile, for each 128-key chunk of the band:
           S^T[keys, 2hx128q] = kT_chunk^T @ qT  (so exp gives P^T directly,
           no PE transposes); masked chunks multiplied by small static 0/1
           patterns; denominator via ones-matmul over partitions; PV
           accumulates enc^T; normalization fused into the PSUM->SBUF move.
  stage C: out partial = encT.T @ wout_slice, interleaved one tile behind
           stage B; yp written as bf16 (host sums partials in fp64).
"""
import sys

sys.path.insert(0, "/opt/trn_rl_repo")

import numpy as np
import ml_dtypes

import concourse.bacc as bacc
import concourse.mybir as mybir
from concourse.bass_utils import run_bass_kernel_spmd
from concourse.tile import TileContext
from concourse.alu_op_type import AluOpType

F32 = mybir.dt.float32
F32R = mybir.dt.float32r
BF16 = mybir.dt.bfloat16
ACTF = mybir.ActivationFunctionType

B, T, WIDTH = 2, 2048, 2048
NUM_HEADS, NUM_KV_HEADS, HEAD_DIM = 8, 4, 256
GROUPS = NUM_HEADS // NUM_KV_HEADS  # 2 q heads per kv head (= per core)
WINDOW = 512
ROPE_BASE = 10000.0
ALPHA = HEAD_DIM ** -0.5

NT = T // 128           # 16 query tiles
TCH = 512               # stage-A t-chunk width
NTCH = T // TCH         # 4
NW = WIDTH // 128       # 16 contraction chunks

_prog_cache = {}
DEBUG_TAPS = False


def _geometry(positions, attn_mask):
    """Per-(tile, chunk) structure from the actual mask/positions data.

    Returns (chunk_lists, slot_of, masks_per_batch):
      chunk_lists[it] = list of 128-key chunk indices with any valid key
                        (union over batches, so the SPMD program is shared)
      slot_of[(it, c)] = mask-pattern slot, or None if fully valid
      masks_per_batch[b] = [n_slots, 128, 256] float32 0/1 multiplicative
                        masks (pattern doubled along axis 1 for the 2 heads)
    """
    pos = np.asarray(positions)
    am = np.asarray(attn_mask)
    pd = pos[:, :, None].astype(np.int64) - pos[:, None, :].astype(np.int64)
    valid = am & (np.abs(pd) < WINDOW)  # [B, T, T] bool
    assert valid.any(axis=2).all(), "a query row with no valid key is unsupported"

    chunk_lists = []
    slot_of = {}
    joint_slots = {}
    slot_blocks = []  # list of [B, 128, 128] bool
    for it in range(NT):
        qs = slice(it * 128, (it + 1) * 128)
        cs = []
        for c in range(NT):
            blk = valid[:, qs, c * 128:(c + 1) * 128]  # [B, 128, 128]
            if not blk.any():
                continue
            cs.append(c)
            if blk.all():
                slot_of[(it, c)] = None
            else:
                key = blk.tobytes()
                if key not in joint_slots:
                    joint_slots[key] = len(slot_blocks)
                    slot_blocks.append(blk)
                slot_of[(it, c)] = joint_slots[key]
        chunk_lists.append(tuple(cs))
    n_slots = max(1, len(slot_blocks))
    assert len(slot_blocks) <= 16, f"too many mask patterns: {len(slot_blocks)}"
    masks = np.ones((B, n_slots, 128, 256), np.float32)
    for s, blk in enumerate(slot_blocks):
        # patterns are collected as [q, key] blocks but applied to the
        # transposed P^T [key, q] tiles -> transpose, doubled for 2 heads
        m = blk.astype(np.float32).transpose(0, 2, 1)  # [B, 128k, 128q]
        masks[:, s, :, 0:128] = m
        masks[:, s, :, 128:256] = m
    return tuple(chunk_lists), slot_of, masks


def _rope_tables(pos_b, scale):
    """cos/sin tables in [head_dim/2, T] (transposed) layout, gain folded in."""
    d = np.arange(HEAD_DIM // 2, dtype=np.float32)
    timescale = (ROPE_BASE ** (2.0 / HEAD_DIM * d)).astype(np.float32)
    rad = pos_b.astype(np.float32)[None, :] / timescale[:, None]  # [128, T]
    cos, sin = np.cos(rad).astype(np.float32), np.sin(rad).astype(np.float32)
    g1 = (1.0 + scale[:HEAD_DIM // 2]).astype(np.float32)[:, None]
    g2 = (1.0 + scale[HEAD_DIM // 2:]).astype(np.float32)[:, None]
    # o1 = a1*C1 - a2*S2 ; o2 = a2*C2 + a1*S1
    return (cos * g1, sin * g1, cos * g2, sin * g2)  # C1, S1, C2, S2


def _build(chunk_lists, slot_items, n_slots, shared_tables, debug_taps=False):
    slot_of = dict(slot_items)
    nc = bacc.Bacc("TRN2", target_bir_lowering=False, debug=False, num_devices=8)

    def din(name, shape, dt):
        return nc.dram_tensor(name, shape, dt, kind="ExternalInput").ap()

    xT = din("xT", [WIDTH, T], BF16)
    wq = din("wq", [WIDTH, 512], BF16)
    wk = din("wk", [WIDTH, 256], BF16)
    wv = din("wv", [WIDTH, 256], BF16)
    wout = din("wout", [512, T], BF16)
    ones1_r = din("ones1", [1, 128], F32R)   # K=1 broadcast lhsT
    onesc_r = din("onesc", [128, 1], F32R)   # partition-sum lhsT (f32r)
    onesc_b = din("onescb", [128, 1], BF16)  # partition-sum lhsT (bf16)
    masks_d = din("masks", [n_slots, 128, 256], BF16)
    tab_names = ["ct", "st"] if shared_tables else [
        "cq1", "sq1", "cq2", "sq2", "ck1", "sk1", "ck2", "sk2"]
    tabs = {n: din(n, [128, T], F32) for n in tab_names}
    yp = nc.dram_tensor("yp", [T, T], BF16, kind="ExternalOutput").ap()

    taps = {}
    if debug_taps:
        for nm, shape in (("qT_tap", [2, 128, NT * 256]), ("kT_tap", [2, 128, T]),
                          ("v_tap", [128, NT * 256]), ("encT_tap", [4, 128, T]),
                          ("pts_tap", [128, 5 * 256])):
            taps[nm] = nc.dram_tensor(nm, shape, BF16, kind="ExternalOutput").ap()

    with TileContext(nc) as tc:
        with (
            tc.tile_pool(name="persist", bufs=1) as pp,
            tc.tile_pool(name="qk_store", bufs=1) as qkp,
        ):
            ones1 = pp.tile([1, 128], F32R)
            nc.sync.dma_start(out=ones1[:], in_=ones1_r[:])
            onescr = pp.tile([128, 1], F32R)
            nc.sync.dma_start(out=onescr[:], in_=onesc_r[:])
            onescb = pp.tile([128, 1], BF16)
            nc.sync.dma_start(out=onescb[:], in_=onesc_b[:])
            masks_sb = pp.tile([128, n_slots * 256], BF16)
            epsb = pp.tile([1, 1], F32)
            nc.any.memset(epsb[:], 1e-6)
            epsbq = pp.tile([1, 1], F32)
            nc.any.memset(epsbq[:], HEAD_DIM * 1e-6)

            # qT layout per cc: [128, (it, h, 128)] so stage-B rhs slices
            # [it*256, it*256+256) pack both q heads contiguously.
            qT = [qkp.tile([128, NT * 256], BF16, tag=f"qT{c}", name=f"qT{c}")
                  for c in range(2)]
            kT = [qkp.tile([128, T], BF16, tag=f"kT{c}", name=f"kT{c}")
                  for c in range(2)]
            v_sb = qkp.tile([128, NT * 256], BF16, tag="vsb", name="vsb")
            encT = [qkp.tile([128, T], BF16, tag=f"encT{j}", name=f"encT{j}")
                    for j in range(4)]
            wout_sb = [qkp.tile([128, T], BF16, tag=f"wo{j}", name=f"wo{j}")
                       for j in range(4)]
            wout_r = wout.rearrange("(c p) t -> c p t", p=128)

            # ---------------- stage A: projections + RMSNorm + RoPE ----------
            with (
                tc.tile_pool(name="wpool", bufs=1) as wp,
                tc.tile_pool(name="xpool", bufs=2) as xp,
                tc.tile_pool(name="tabpool", bufs=2) as tp,
                tc.tile_pool(name="sa", bufs=2) as sa,
                tc.tile_pool(name="psA", bufs=2, space="PSUM") as psA,
                tc.tile_pool(name="psAux", bufs=2, space="PSUM") as psAux,
                tc.tile_pool(name="psAux1", bufs=1, space="PSUM") as psAux1,
            ):
                wq_t = wp.tile([128, NW * 512], BF16)
                wk_t = wp.tile([128, NW * 256], BF16)
                wv_t = wp.tile([128, NW * 256], BF16)
                wq_r = wq.rearrange("(c p) m -> p c m", p=128)
                wk_r = wk.rearrange("(c p) m -> p c m", p=128)
                wv_r = wv.rearrange("(c p) m -> p c m", p=128)
                wq_v = wq_t[:].rearrange("p (c m) -> p c m", m=512)
                wk_v = wk_t[:].rearrange("p (c m) -> p c m", m=256)
                wv_v = wv_t[:].rearrange("p (c m) -> p c m", m=256)
                xT_r = xT.rearrange("(c p) t -> p c t", p=128)

                def load_xts(tci):
                    t0 = tci * TCH
                    xts = xp.tile([128, NW * TCH], BF16, tag="xts", name=f"xts{tci}")
                    xv = xts[:].rearrange("p (c t) -> p c t", t=TCH)
                    for q4 in range(4):
                        nc.sync.dma_start(
                            out=xv[:, q4 * 4:(q4 + 1) * 4],
                            in_=xT_r[:, q4 * 4:(q4 + 1) * 4, t0:t0 + TCH],
                        )
                    return xts

                xts_pre = xp.tile([128, NW * TCH], BF16, tag="xts", name="xts0")
                xv0 = xts_pre[:].rearrange("p (c t) -> p c t", t=TCH)
                for q4 in range(4):
                    nc.sync.dma_start(out=wq_v[:, q4 * 4:(q4 + 1) * 4],
                                      in_=wq_r[:, q4 * 4:(q4 + 1) * 4])
                    nc.sync.dma_start(out=xv0[:, q4 * 4:(q4 + 1) * 4],
                                      in_=xT_r[:, q4 * 4:(q4 + 1) * 4, 0:TCH])
                for q4 in range(4):
                    nc.sync.dma_start(out=wk_v[:, q4 * 4:(q4 + 1) * 4],
                                      in_=wk_r[:, q4 * 4:(q4 + 1) * 4])

                if shared_tables:
                    q_tabs = k_tabs = ("ct", "st", "ct", "st")
                else:
                    q_tabs = ("cq1", "sq1", "cq2", "sq2")
                    k_tabs = ("ck1", "sk1", "ck2", "sk2")
                # (w_tile, wcols, col_base, table_keys, is_q, qhead)
                # q-head1 last: its final-chunk rope runs on GPSIMD after
                # stage A ends and only gates stage-B tiles 12-15
                units = [
                    (wq_t, 512, 0, q_tabs, True, 0),
                    (wk_t, 256, 0, k_tabs, False, None),
                    (wq_t, 512, 256, q_tabs, True, 1),
                ]

                def emit_v(tci, s4, xts):
                    t0 = tci * TCH
                    psv = psAux.tile([128, 256], F32, tag="pv",
                                     name=f"psv{tci}_{s4}")
                    for wc in range(NW):
                        nc.tensor.matmul(
                            psv[:],
                            xts[:, wc * TCH + s4 * 128: wc * TCH + (s4 + 1) * 128],
                            wv_t[:, wc * 256:(wc + 1) * 256],
                            start=(wc == 0), stop=(wc == NW - 1),
                        )
                    it = tci * 4 + s4
                    nc.scalar.activation(v_sb[:, it * 256:(it + 1) * 256],
                                         psv[:], ACTF.Copy)

                def emit_aux(pend):
                    """Variance -> rstd broadcast -> RoPE for a finished unit."""
                    (tci, ps1, ps2, tabt, tkeys, is_q, qhead) = pend
                    t0 = tci * TCH
                    sq1 = sa.tile([128, TCH], F32R, tag="sq1")
                    sq2 = sa.tile([128, TCH], F32R, tag="sq2")
                    nc.scalar.activation(sq1[:], ps1[:], ACTF.Square)
                    nc.scalar.activation(sq2[:], ps2[:], ACTF.Square)
                    psvar = psAux1.tile([1, TCH], F32, tag="var")
                    nc.tensor.matmul(psvar[:], onescr[:], sq1[:], start=True, stop=False)
                    nc.tensor.matmul(psvar[:], onescr[:], sq2[:], start=False, stop=True)
                    stdv = sa.tile([1, TCH], F32R, tag="stdv")
                    if is_q:
                        # fold alpha: stdv = sqrt(sum(q^2) + 256*eps) = 16*sqrt(var+eps)
                        nc.scalar.activation(stdv[:], psvar[:], ACTF.Sqrt,
                                             bias=epsbq[:])
                    else:
                        nc.scalar.activation(stdv[:], psvar[:], ACTF.Sqrt,
                                             scale=1.0 / HEAD_DIM, bias=epsb[:])
                    # evacuate the projection PSUM banks promptly via ACT so
                    # the next pipeline stage (and stage B's pools at the A->B
                    # boundary) can claim them without waiting on DVE's tail
                    cs1 = sa.tile([128, TCH], F32, tag="cs1")
                    cs2 = sa.tile([128, TCH], F32, tag="cs2")
                    nc.scalar.activation(cs1[:], ps1[:], ACTF.Copy)
                    nc.scalar.activation(cs2[:], ps2[:], ACTF.Copy)
                    psb = psAux1.tile([128, TCH], F32, tag="bc")
                    nc.tensor.matmul(psb[:], ones1[:], stdv[:], start=True, stop=True)
                    rb = sa.tile([128, TCH], F32, tag="rb")
                    nc.vector.reciprocal_approx_fast(out=rb[:], in_=psb[:])
                    # the very last unit's rope runs post-stage-A with nothing
                    # to hide it on DVE; GPSIMD is idle then (SBUF-only ops)
                    eng = (nc.gpsimd if (tci == NTCH - 1 and qhead == 1)
                           else nc.vector)
                    a1 = sa.tile([128, TCH], F32, tag="a1")
                    a2 = sa.tile([128, TCH], F32, tag="a2")
                    eng.tensor_tensor(a1[:], cs1[:], rb[:], AluOpType.mult)
                    eng.tensor_tensor(a2[:], cs2[:], rb[:], AluOpType.mult)
                    C1, S1, C2, S2 = (tabt[k] for k in tkeys)
                    m1 = sa.tile([128, TCH], F32, tag="m1")
                    m2 = sa.tile([128, TCH], F32, tag="m2")
                    m3 = sa.tile([128, TCH], F32, tag="m1", name="m3t")
                    m4 = sa.tile([128, TCH], F32, tag="m2", name="m4t")
                    eng.tensor_tensor(m1[:], a1[:], C1[:], AluOpType.mult)
                    eng.tensor_tensor(m2[:], a2[:], S2[:], AluOpType.mult)
                    eng.tensor_tensor(m3[:], a2[:], C2[:], AluOpType.mult)
                    eng.tensor_tensor(m4[:], a1[:], S1[:], AluOpType.mult)
                    if is_q:
                        qv1 = qT[0][:].rearrange("p (i m) -> p i m", m=256)
                        qv2 = qT[1][:].rearrange("p (i m) -> p i m", m=256)
                        d1 = qv1[:, tci * 4:(tci + 1) * 4, qhead * 128:(qhead + 1) * 128]
                        d2 = qv2[:, tci * 4:(tci + 1) * 4, qhead * 128:(qhead + 1) * 128]
                    else:
                        d1 = kT[0][:].rearrange("p (i m) -> p i m", m=128)[
                            :, tci * 4:(tci + 1) * 4, :]
                        d2 = kT[1][:].rearrange("p (i m) -> p i m", m=128)[
                            :, tci * 4:(tci + 1) * 4, :]
                    m1v = m1[:].rearrange("p (i m) -> p i m", m=128)
                    m2v = m2[:].rearrange("p (i m) -> p i m", m=128)
                    m3v = m3[:].rearrange("p (i m) -> p i m", m=128)
                    m4v = m4[:].rearrange("p (i m) -> p i m", m=128)
                    eng.tensor_tensor(d1, m1v, m2v, AluOpType.subtract)
                    eng.tensor_tensor(d2, m3v, m4v, AluOpType.add)

                pend = None
                for tci in range(NTCH):
                    t0 = tci * TCH
                    xts = xts_pre if tci == 0 else load_xts(tci)
                    tabt = {}
                    for name in dict.fromkeys(q_tabs + k_tabs):
                        tt = tp.tile([128, TCH], F32, tag=name, name=f"tab_{name}")
                        nc.sync.dma_start(out=tt[:], in_=tabs[name][:, t0:t0 + TCH])
                        tabt[name] = tt
                    if tci == 0:
                        for q4 in range(4):
                            nc.sync.dma_start(out=wv_v[:, q4 * 4:(q4 + 1) * 4],
                                              in_=wv_r[:, q4 * 4:(q4 + 1) * 4])
                        # stage-B/C-only data: queued after the stage-A kickoff
                        # so it doesn't delay the first matmuls
                        nc.sync.dma_start(
                            out=masks_sb[:].rearrange("p (s m) -> p s m", m=256),
                            in_=masks_d.rearrange("s p m -> p s m"))
                        for j in range(4):
                            nc.sync.dma_start(out=wout_sb[j][:], in_=wout_r[j])
                    for ui, (w_t, wcols, cbase, tkeys, is_q, qhead) in enumerate(units):
                        ps1 = psA.tile([128, TCH], F32, tag="s0")
                        ps2 = psA.tile([128, TCH], F32, tag="s1")
                        for ps, cc in ((ps1, 0), (ps2, 1)):
                            coff = cbase + cc * 128
                            for wc in range(NW):
                                nc.tensor.matmul(
                                    ps[:],
                                    w_t[:, wc * wcols + coff: wc * wcols + coff + 128],
                                    xts[:, wc * TCH:(wc + 1) * TCH],
                                    start=(wc == 0), stop=(wc == NW - 1),
                                )
                        # v-projection matmuls act as PE filler while ACT/DVE
                        # work through the previous unit's variance/rope chain.
                        emit_v(tci, ui, xts)
                        if pend is not None:
                            emit_aux(pend)
                        pend = (tci, ps1, ps2, tabt, tkeys, is_q, qhead)
                    emit_v(tci, 3, xts)
                emit_aux(pend)

            if debug_taps:
                for c in range(2):
                    nc.sync.dma_start(out=taps["qT_tap"][c], in_=qT[c][:])
                    nc.sync.dma_start(out=taps["kT_tap"][c], in_=kT[c][:])
                nc.sync.dma_start(out=taps["v_tap"][:], in_=v_sb[:])

            # ---------------- stage B + C: banded attention + out proj -------
            with (
                tc.tile_pool(name="ptsp", bufs=2) as ptsp,
                tc.tile_pool(name="sb", bufs=2) as sbp,
                tc.tile_pool(name="outp", bufs=4) as outp,
                tc.tile_pool(name="psS", bufs=3, space="PSUM") as psS,
                tc.tile_pool(name="psDen", bufs=1, space="PSUM") as psDen,
                tc.tile_pool(name="psE", bufs=1, space="PSUM") as psE,
                tc.tile_pool(name="psO", bufs=2, space="PSUM") as psO,
            ):
                def emit_C(it):
                    for nb in range(4):
                        ops = psO.tile([128, 512], F32, tag="o", name=f"o{it}_{nb}")
                        for j in range(4):
                            nc.tensor.matmul(
                                ops[:],
                                encT[j][:, it * 128:(it + 1) * 128],
                                wout_sb[j][:, nb * 512:(nb + 1) * 512],
                                start=(j == 0), stop=(j == 3),
                            )
                        ob = outp.tile([128, 512], BF16, tag="ob", name=f"ob{it}_{nb}")
                        if nb % 2 == 0:
                            nc.scalar.activation(ob[:], ops[:], ACTF.Copy)
                        else:
                            nc.vector.tensor_copy(ob[:], ops[:])
                        nc.sync.dma_start(
                            out=yp[it * 128:(it + 1) * 128, nb * 512:(nb + 1) * 512],
                            in_=ob[:])

                pendC = None

                def tile_body(it):
                    nonlocal pendC
                    cs = chunk_lists[it]
                    ncs = len(cs)
                    pts = ptsp.tile([128, ncs * 256], BF16, tag="pts", name=f"pts{it}")
                    # S^T per chunk + exp (+ mask)
                    for ci, c in enumerate(cs):
                        pss = psS.tile([128, 256], F32, tag="s", name=f"s{it}_{ci}")
                        for cc in range(2):
                            nc.tensor.matmul(
                                pss[:],
                                kT[cc][:, c * 128:(c + 1) * 128],
                                qT[cc][:, it * 256:(it + 1) * 256],
                                start=(cc == 0), stop=(cc == 1),
                            )
                        slot = slot_of[(it, c)]
                        dst = pts[:, ci * 256:(ci + 1) * 256]
                        if slot is None:
                            nc.scalar.activation(dst, pss[:], ACTF.Exp)
                        else:
                            ptmp = sbp.tile([128, 256], BF16, tag="ptmp",
                                            name=f"pt{it}_{ci}")
                            nc.scalar.activation(ptmp[:], pss[:], ACTF.Exp)
                            # GPSIMD (idle otherwise) so the DVE queue never
                            # gates the denominator matmuls
                            nc.gpsimd.tensor_tensor(
                                dst, ptmp[:],
                                masks_sb[:, slot * 256:(slot + 1) * 256],
                                AluOpType.mult)
                    # stage-C matmuls of the previous tile fill the PE queue
                    # while ACT finishes the exps this tile's den/PV need.
                    if pendC is not None:
                        emit_C(pendC)
                    # denominator (partition-sum of P^T over all chunks)
                    psd = psDen.tile([1, 256], F32, tag="dn", name=f"d{it}")
                    for ci in range(ncs):
                        nc.tensor.matmul(
                            psd[:], onescb[:], pts[:, ci * 256:(ci + 1) * 256],
                            start=(ci == 0), stop=(ci == ncs - 1),
                        )
                    den_s = sbp.tile([1, 256], F32R, tag="dens", name=f"dens{it}")
                    nc.scalar.activation(den_s[:], psd[:], ACTF.Copy)
                    # PV accumulation (hides the recip turnaround)
                    pse = [psE.tile([128, 256], F32, tag=f"e{cc}", name=f"e{it}_{cc}")
                           for cc in range(2)]
                    for cc in range(2):
                        for ci, c in enumerate(cs):
                            nc.tensor.matmul(
                                pse[cc][:],
                                v_sb[:, c * 256 + cc * 128: c * 256 + (cc + 1) * 128],
                                pts[:, ci * 256:(ci + 1) * 256],
                                start=(ci == 0), stop=(ci == ncs - 1),
                            )
                    psrb = psDen.tile([128, 256], F32, tag="dn", name=f"rb{it}")
                    nc.tensor.matmul(psrb[:], ones1[:], den_s[:],
                                     start=True, stop=True)
                    # normalize enc^T into SBUF (overlaps with next tile's S^T)
                    rdenb = sbp.tile([128, 256], F32, tag="rdenb", name=f"rdb{it}")
                    nc.vector.reciprocal_approx_fast(out=rdenb[:], in_=psrb[:])
                    for cc in range(2):
                        for h in range(2):
                            nc.vector.tensor_tensor(
                                encT[2 * h + cc][:, it * 128:(it + 1) * 128],
                                pse[cc][:, h * 128:(h + 1) * 128],
                                rdenb[:, h * 128:(h + 1) * 128],
                                AluOpType.mult)
                    if debug_taps and it == 8:
                        nc.sync.dma_start(out=taps["pts_tap"][:], in_=pts[:])
                    pendC = it

                for it in range(NT):
                    tile_body(it)
                emit_C(pendC)

                if debug_taps:
                    for j in range(4):
                        nc.sync.dma_start(out=taps["encT_tap"][j], in_=encT[j][:])

    nc.compile()
    return nc


def kernel(x, positions, attn_mask, wq, wkv, wout, q_scale, k_scale):
    x = np.ascontiguousarray(x, np.float32)
    positions = np.asarray(positions)
    wq = np.ascontiguousarray(wq, np.float32)
    wkv = np.ascontiguousarray(wkv, np.float32)
    wout = np.ascontiguousarray(wout, np.float32)
    q_scale = np.asarray(q_scale, np.float32)
    k_scale = np.asarray(k_scale, np.float32)

    chunk_lists, slot_of, masks = _geometry(positions, attn_mask)
    n_slots = masks.shape[1]
    shared = not (q_scale.any() or k_scale.any())

    key = (chunk_lists, tuple(sorted(slot_of.items(), key=lambda kv: kv[0])),
           n_slots, shared, DEBUG_TAPS)
    if key not in _prog_cache:
        _prog_cache[key] = _build(chunk_lists, key[1], n_slots, shared, DEBUG_TAPS)
    nc = _prog_cache[key]

    bf = ml_dtypes.bfloat16
    ones1 = np.ones((1, 128), np.float32)
    onesc = np.ones((128, 1), np.float32)

    in_maps = []
    for core in range(8):
        b, kh = divmod(core, NUM_KV_HEADS)
        m = {
            "xT": np.ascontiguousarray(x[b].T).astype(bf),
            "wq": np.ascontiguousarray(wq[:, kh * 512:(kh + 1) * 512]).astype(bf),
            "wk": np.ascontiguousarray(wkv[:, kh * 256:(kh + 1) * 256]).astype(bf),
            "wv": np.ascontiguousarray(
                wkv[:, 1024 + kh * 256: 1024 + (kh + 1) * 256]).astype(bf),
            "wout": np.ascontiguousarray(wout[kh * 512:(kh + 1) * 512, :]).astype(bf),
            "ones1": ones1, "onesc": onesc, "onescb": onesc.astype(bf),
            "masks": masks[b].astype(bf),
        }
        if shared:
            ct, st, _, _ = _rope_tables(positions[b], np.zeros(HEAD_DIM, np.float32))
            m["ct"], m["st"] = ct, st
        else:
            for nm, tb in zip(("cq1", "sq1", "cq2", "sq2"),
                              _rope_tables(positions[b], q_scale)):
                m[nm] = tb
            for nm, tb in zip(("ck1", "sk1", "ck2", "sk2"),
                              _rope_tables(positions[b], k_scale)):
                m[nm] = tb
        in_maps.append(m)

    res = run_bass_kernel_spmd(nc, in_maps, list(range(8)))
    kernel._last_results = res
    out = np.empty((B, T, T), np.float32)
    for b in range(B):
        acc = res.results[b * NUM_KV_HEADS]["yp"].astype(np.float64)
        for kh in range(1, NUM_KV_HEADS):
            acc += res.results[b * NUM_KV_HEADS + kh]["yp"].astype(np.float64)
        out[b] = acc.astype(np.float32)
    return out
